# revision 47
# baseline (speedup 1.0000x reference)
"""Trainium2 Bass kernel for nn_CHConv — windowed deformable 3x3 conv, v5.

Design notes:
- Geometry (offsets/windows/weights) is image-independent. Shard by
  (cm = h%4, xh = x-half): core = cm*2+xh handles rows {4j+cm}, columns
  [xh*128, xh*128+128), for BOTH images. The two images share one weight
  read (halves the dominant weight DMA vs batch-sharding).
- The core's column slice of BOTH images stays resident in SBUF,
  j-duplicated: partition (j*64+c) holds x[img, row, col+j, c] for a
  [136 row x 146 col] padded window. One chunk = one row-group; windowed
  taps read strided windows straight from the resident tile.
- T-build: one 5-dim DVE tensor_tensor per tap covers both images
  (weights read once, img dim stride-0 on the weight AP).
- Seam taps (equirect wrap, dx ~ +204..255) use gpsimd.dma_gather with
  1024B elements carrying BOTH images' 2x2xC patch; gathers are grouped
  8 chunks per call (padding only per group) to cut SWDGE descriptor
  generation cost.
- Matmuls: k-major accumulation into one PSUM bank [128F, 256=(img,x)]
  per chunk; wrap matmuls land on column intervals. ACT copies PSUM out
  as bf16.
"""
import numpy as np
from contextlib import ExitStack

import concourse.bass as bass
import concourse.bacc as bacc
import concourse.mybir as mybir
import concourse.tile as tile
from concourse.bass_utils import run_bass_kernel_spmd
from ml_dtypes import bfloat16

H, W, K, C, F, B = 128, 256, 9, 64, 128, 2
NCH = 32            # chunks (row-groups) per core
GSZ = 8             # chunks per gather group
NG = NCH // GSZ     # gather groups
XW = 128            # x columns per core
RROWS = 136         # resident rows: image rows [cm-4, cm+132)
RCOLS = 146         # resident cols: image cols [xh*128-8, xh*128+138)
HALO = 8
A_MAX = 10
NPIX = H * W
NCORES = 8
RSZ = RROWS * RCOLS

_BF16 = mybir.dt.bfloat16
_F32 = mybir.dt.float32
_I16 = mybir.dt.int16


# ---------------------------------------------------------------- host plan
def geometry(scale, offset_base):
    off = (offset_base.astype(np.float32) * scale.astype(np.float32)).reshape(
        H, W, K, 2)
    ti, tj = np.meshgrid(np.arange(3), np.arange(3), indexing="ij")
    ti = ti.reshape(-1).astype(np.float32)
    tj = tj.reshape(-1).astype(np.float32)
    ys = (np.arange(H, dtype=np.float32)[:, None, None] - 1.0 + ti[None, None]
          + off[..., 0])
    xs = (np.arange(W, dtype=np.float32)[None, :, None] - 1.0 + tj[None, None]
          + off[..., 1])
    y0 = np.floor(ys); x0 = np.floor(xs)
    fy = (ys - y0).astype(np.float32); fx = (xs - x0).astype(np.float32)
    y0 = y0.astype(np.int64); x0 = x0.astype(np.int64)

    def v(yi, xi):
        return (((yi >= 0) & (yi < H) & (xi >= 0) & (xi < W))
                .astype(np.float32))
    w = np.zeros((H, W, K, 2, 2), np.float32)
    w[..., 0, 0] = (1 - fy) * (1 - fx) * v(y0, x0)
    w[..., 0, 1] = (1 - fy) * fx * v(y0, x0 + 1)
    w[..., 1, 0] = fy * (1 - fx) * v(y0 + 1, x0)
    w[..., 1, 1] = fy * fx * v(y0 + 1, x0 + 1)
    return y0, x0, w


def make_plan(y0, x0, w):
    """Per-chunk meta shared by all 8 cores (union over 4 rows x 2 halves)."""
    live = w.sum(axis=(3, 4)) > 0
    dy = y0 - np.arange(H)[:, None, None]
    dx = x0 - np.arange(W)[None, :, None]
    plan = []
    for ci in range(NCH):
        rows = [4 * ci + r for r in range(4)]
        win, wrap = [], []
        for k in range(K):
            lv = np.stack([live[h, :, k] for h in rows])
            if not lv.any():
                continue
            dyr = np.stack([dy[h, :, k] for h in rows])
            dxr = np.stack([dx[h, :, k] for h in rows])
            dmn, dmx = int(dxr[lv].min()), int(dxr[lv].max())
            ymn, ymx = int(dyr[lv].min()), int(dyr[lv].max())
            awid, bwid = ymx - ymn + 2, dmx - dmn + 2
            amin, bmin = ymn, dmn
            ok = (bmin >= -HALO and bmin + bwid <= HALO + 3
                  and awid <= A_MAX
                  and 4 * ci + 4 + amin >= 0
                  and 4 * ci + 4 + amin + awid <= RROWS)
            if ok:
                win.append(dict(k=k, amin=amin, awid=awid, bmin=bmin,
                                bwid=bwid, nslot=awid * (bwid - 1)))
            else:
                ws = np.where(lv.any(axis=0))[0]
                glo, ghi = int(ws.min()), int(ws.max())
                lo, hi = 127, 0
                for xh in range(2):
                    l_ = max(glo - 128 * xh, 0)
                    h_ = min(ghi - 128 * xh, 127)
                    if l_ <= h_:
                        lo = min(lo, l_); hi = max(hi, h_)
                assert lo <= hi
                wrap.append(dict(k=k, lo=lo, wd=hi - lo + 1))
        assert win, f"chunk {ci} has no windowed tap"
        soff = 0
        for t in win:
            t["soff"] = soff
            soff += t["nslot"]
        nw = sum(t["wd"] for t in wrap)
        noff = 0
        for t in wrap:
            t["noff"] = noff
            noff += t["wd"]
        plan.append(dict(ci=ci, win=win, wrap=wrap, nslots=soff, nw=nw,
                         wfree=soff * XW))
    # processing order: interior chunks first so the gather-heavy pole
    # chunks (0-2) overlap with the bulk of the compute.
    interior = [ci for ci in range(NCH) if plan[ci]["nw"] <= 400]
    poles = [ci for ci in range(NCH) if plan[ci]["nw"] > 400]
    cut = (2 * len(interior)) // 3
    order = interior[:cut] + poles + interior[cut:]
    # gather groups: adaptive over processing order — big-gather chunks go
    # solo, interior chunks pack greedily up to ~1200 idxs.
    groups = []
    cur, cur_nw = [], 0
    for ci in order:
        ch = plan[ci]
        big = ch["nw"] > 400
        if cur and (big or cur_nw + ch["nw"] > 450):
            groups.append(cur)
            cur, cur_nw = [], 0
        cur.append(ci)
        cur_nw += ch["nw"]
        if big or len(cur) >= 3:
            groups.append(cur)
            cur, cur_nw = [], 0
    if cur:
        groups.append(cur)
    gmeta = []
    for g, chs in enumerate(groups):
        off = 0
        for ci in chs:
            plan[ci]["goff"] = off
            plan[ci]["grp"] = g
            off += plan[ci]["nw"]
        ngg = ((off + 127) // 128) * 128 if off else 0
        gmeta.append(dict(g=g, ngg=ngg, raw=off, chunks=tuple(chs)))
    return plan, gmeta, order


def plan_sig(plan, groups, order):
    sig = []
    for ch in plan:
        sig.append((ch["nslots"], ch["nw"], ch["goff"], ch["grp"],
                    tuple((t["k"], t["amin"], t["awid"], t["bmin"], t["bwid"])
                          for t in ch["win"]),
                    tuple((t["k"], t["lo"], t["wd"]) for t in ch["wrap"])))
    sig.append(tuple((gr["ngg"], gr["chunks"]) for gr in groups))
    sig.append(tuple(order))
    return tuple(sig)


# ---------------------------------------------------------------- bass build
def build_bass(plan, groups, order):
    totw = sum(ch["wfree"] for ch in plan)
    totng = sum(gr["ngg"] for gr in groups)
    goffs = np.cumsum([0] + [gr["ngg"] for gr in groups])[:-1]

    nc = bacc.Bacc("TRN2", target_bir_lowering=False, debug=False)
    resid = nc.dram_tensor("resid", [2, 128, RSZ], _BF16,
                           kind="ExternalInput")          # [img, (j,c), r*c]
    wcomp = nc.dram_tensor("wcomp", [128, totw], _BF16, kind="ExternalInput")
    wgr = nc.dram_tensor("wgr", [128, 2 * totng], _BF16,
                         kind="ExternalInput")            # per-group [i2, ngg]
    widx = nc.dram_tensor("widx", [128, max(totng // 16, 16)], _I16,
                          kind="ExternalInput")
    xdp = nc.dram_tensor("xdp", [NPIX, 8 * C], _BF16, kind="ExternalInput")
    kdup = nc.dram_tensor("kdup", [128, K * F], _BF16, kind="ExternalInput")
    out = nc.dram_tensor("out", [F, NCH * 256], _BF16, kind="ExternalOutput")

    woffs = np.cumsum([0] + [ch["wfree"] for ch in plan])[:-1]

    with ExitStack() as ctx:
        tc = ctx.enter_context(tile.TileContext(nc))
        rp = ctx.enter_context(tc.tile_pool(name="rp", bufs=1))
        kp = ctx.enter_context(tc.tile_pool(name="kp", bufs=1))
        idxp = ctx.enter_context(tc.tile_pool(name="idxp", bufs=1))
        wbp = ctx.enter_context(tc.tile_pool(name="wbp", bufs=6))
        wgp = ctx.enter_context(tc.tile_pool(name="wgp", bufs=3))
        twp = ctx.enter_context(tc.tile_pool(name="twp", bufs=2))
        gtp = ctx.enter_context(tc.tile_pool(name="gtp", bufs=5))
        trp = ctx.enter_context(tc.tile_pool(name="trp", bufs=1))
        op_ = ctx.enter_context(tc.tile_pool(name="op", bufs=3))
        max_open = max(
            len(groups[i]["chunks"]) + (len(groups[i + 1]["chunks"])
                                        if i + 1 < len(groups) else 0)
            for i in range(len(groups)))
        psp = ctx.enter_context(tc.tile_pool(name="psp",
                                             bufs=min(max_open + 1, 8),
                                             space="PSUM"))

        kd = kp.tile([128, K * F], _BF16)
        nc.sync.dma_start(out=kd[:], in_=kdup[:, :])
        idx_t = idxp.tile([128, totng // 16], _I16)
        nc.sync.dma_start(out=idx_t[:], in_=widx[:, 0:totng // 16])
        res = rp.tile([128, 2 * RSZ], _BF16)
        # band-split resident load (17-row bands) so early chunks can start
        # as soon as their rows land; bands follow chunk processing order.
        BAND = 17
        nbands = (RROWS + BAND - 1) // BAND
        for b in range(nbands):
            r0, r1 = b * BAND, min((b + 1) * BAND, RROWS)
            for m in range(2):
                nc.sync.dma_start(
                    out=res[:, m * RSZ + r0 * RCOLS:m * RSZ + r1 * RCOLS],
                    in_=resid[m, :, r0 * RCOLS:r1 * RCOLS])

        wbs, gts, wgs, trs = {}, {}, {}, {}

        def stage_load(t):
            ch = plan[t]
            wbs[t] = wbp.tile([128, ch["wfree"]], _BF16, name=f"wb{t}",
                              tag="wb")
            nc.scalar.dma_start(out=wbs[t][:],
                                in_=wcomp[:, woffs[t]:woffs[t] + ch["wfree"]])

        def stage_gather(g):
            ngg = groups[g]["ngg"]
            if not ngg:
                return
            gts[g] = gtp.tile([128, 4, ngg], _BF16, name=f"gt{g}", tag="gt")
            wgs[g] = wgp.tile([128, 2, ngg], _BF16, name=f"wg{g}", tag="wg")
            go = int(goffs[g])
            nc.scalar.dma_start(out=wgs[g][:],
                                in_=wgr[:, 2 * go:2 * (go + ngg)])
            nc.gpsimd.dma_gather(
                out_ap=gts[g][:], in_ap=xdp[:, :],
                idxs_ap=idx_t[:, go // 16:(go + ngg) // 16],
                num_idxs=ngg, num_idxs_reg=ngg,
                elem_size=8 * C, transpose=True, single_packet=False,
            )

        def stage_wrap_weight(g):
            ngg = groups[g]["ngg"]
            if not ngg:
                return
            gt, wg = gts.pop(g), wgs.pop(g)
            t4 = trp.tile([128, 4, ngg], _BF16, name=f"t4{g}", tag="t4")
            w4 = bass.AP(wg.tensor, wg.offset,
                         [[wg.ap[0][0], 128], [0, 2], [ngg, 2], [1, ngg]])
            nc.vector.tensor_tensor(out=t4[:], in0=gt[:], in1=w4,
                                    op=mybir.AluOpType.mult)
            tr = trp.tile([128, 2, ngg], _BF16, name=f"tr{g}", tag="tr")
            t4v = t4[:].rearrange("p (m i) n -> p m i n", m=2)
            nc.vector.tensor_tensor(
                out=tr[:], in0=t4v[:, :, 0, :], in1=t4v[:, :, 1, :],
                op=mybir.AluOpType.add)
            trs[g] = tr

        ps_open = {}

        def stage_win(t):
            ch = plan[t]
            wb = wbs.pop(t)
            nslots = ch["nslots"]
            tw = twp.tile([128, nslots, 256], _BF16, name=f"tw{t}", tag="tw")
            ps = psp.tile([128, 256], _F32, space="PSUM", name=f"ps{t}",
                          tag="ps")
            ps_open[t] = ps
            ps = ps[:]

            rbase = res[:]
            for tp in ch["win"]:
                a0, b0 = tp["amin"], tp["bmin"]
                aw_, bw_ = tp["awid"], tp["bwid"]
                s0 = tp["soff"]
                if bw_ == 2:
                    src4 = bass.AP(
                        rbase.tensor,
                        rbase.offset + (4 * t + 4 + a0) * RCOLS + HALO + b0,
                        [[rbase.ap[0][0], 128], [RCOLS, aw_], [RSZ, 2],
                         [1, XW]],
                    )
                    wsl = bass.AP(
                        wb.tensor,
                        wb.offset + s0 * XW,
                        [[wb.ap[0][0], 128], [XW, aw_], [0, 2], [1, XW]],
                    )
                    dst4 = bass.AP(
                        tw.tensor,
                        tw.offset + s0 * 256,
                        [[tw.ap[0][0], 128], [256, aw_], [XW, 2], [1, XW]],
                    )
                    nc.vector.tensor_tensor(out=dst4, in0=src4, in1=wsl,
                                            op=mybir.AluOpType.mult)
                    continue
                wsl = bass.AP(
                    wb.tensor,
                    wb.offset + s0 * XW,
                    [[wb.ap[0][0], 128], [XW * (bw_ - 1), aw_],
                     [XW, bw_ - 1], [1, XW]],
                )
                for m in range(2):
                    src4 = bass.AP(
                        rbase.tensor,
                        rbase.offset + m * RSZ
                        + (4 * t + 4 + a0) * RCOLS + HALO + b0,
                        [[rbase.ap[0][0], 128], [RCOLS, aw_], [1, bw_ - 1],
                         [1, XW]],
                    )
                    dst4 = bass.AP(
                        tw.tensor,
                        tw.offset + s0 * 256 + m * XW,
                        [[tw.ap[0][0], 128], [256 * (bw_ - 1), aw_],
                         [256, bw_ - 1], [1, XW]],
                    )
                    nc.vector.tensor_tensor(out=dst4, in0=src4, in1=wsl,
                                            op=mybir.AluOpType.mult)

            mm_list = []
            for tp in ch["win"]:
                for sx in range(tp["nslot"]):
                    mm_list.append((tp["k"], tp["soff"] + sx))
            mm_list.sort()
            has_wrap = bool(ch["wrap"])
            last = len(mm_list) - 1
            for i, (k, s) in enumerate(mm_list):
                nc.tensor.matmul(
                    ps, lhsT=kd[:, k * F:(k + 1) * F],
                    rhs=tw[:, s, :], start=(i == 0),
                    stop=(i == last and not has_wrap))

        def finish_group(g):
            stage_wrap_weight(g)
            tr = trs.pop(g, None)
            chs = groups[g]["chunks"]
            for idx, t in enumerate(chs):
                ch = plan[t]
                pst = ps_open.pop(t)
                ps = pst[:]
                goff_c = ch.get("goff", 0)
                wr_list = []
                for tp in ch["wrap"]:
                    for m in range(2):
                        wr_list.append((tp["k"], tp["lo"], tp["wd"],
                                        tp["noff"], m))
                last = len(wr_list) - 1
                for i, (k, lo, wd, noff, m) in enumerate(wr_list):
                    nc.tensor.matmul(
                        ps[:, m * XW + lo:m * XW + lo + wd],
                        lhsT=kd[:, k * F:(k + 1) * F],
                        rhs=tr[:, m, goff_c + noff:goff_c + noff + wd],
                        start=False, stop=(i == last))
                ob = op_.tile([128, 256], _BF16, name=f"ob{t}", tag="ob")
                nc.scalar.copy(out=ob[:], in_=pst[:])
                nc.scalar.dma_start(out=out[:, t * 256:(t + 1) * 256],
                                    in_=ob[:])

        for i in range(min(6, NCH)):
            stage_load(order[i])
        for g in range(min(3, len(groups))):
            stage_gather(g)
        pos = 0
        for gi, gr in enumerate(groups):
            if gi + 3 < len(groups):
                stage_gather(gi + 3)
            for t in gr["chunks"]:
                if pos + 6 < NCH:
                    stage_load(order[pos + 6])
                stage_win(t)
                pos += 1
            if gi >= 1:
                finish_group(gi - 1)
        finish_group(len(groups) - 1)
    nc.finalize()
    return nc


# ---------------------------------------------------------------- host pack
def pack_inputs(x, kern, plan, groups, y0a, x0a, wts):
    xbf = x.astype(np.float32)
    xp = np.zeros((B, 140, 275, C), np.float32)
    xp[:, 4:4 + H, 8:8 + W, :] = xbf
    xp = xp.astype(bfloat16)

    xpad = np.pad(xbf, [(0, 0), (0, 1), (0, 1), (0, 0)])
    xd = np.empty((H, W, 2, 2, 2, C), np.float32)
    for i in range(2):
        for j in range(2):
            xd[:, :, :, i, j, :] = xpad[:, i:i + H, j:j + W, :].transpose(
                1, 2, 0, 3)
    xdp = xd.reshape(NPIX, 8 * C).astype(bfloat16)

    km = kern.reshape(F, C, K).astype(np.float32)
    kdv = km.transpose(1, 2, 0).reshape(C, K * F)
    kdup = np.concatenate([kdv, kdv], axis=0).astype(bfloat16)

    totw = sum(ch["wfree"] for ch in plan)
    totng = sum(gr["ngg"] for gr in groups)
    goffs = np.cumsum([0] + [gr["ngg"] for gr in groups])[:-1]

    gy = np.clip(y0a, 0, H - 2)
    gx = np.clip(x0a, 0, W - 2)
    pidx = (gy * W + gx).astype(np.int32)
    wcell = np.zeros((H, W, K, 2, 2), np.float32)
    for a_ in range(2):
        for b_ in range(2):
            for i in range(2):
                for j in range(2):
                    m = ((y0a + a_) == (gy + i)) & ((x0a + b_) == (gx + j))
                    wcell[..., i, j] += wts[..., a_, b_] * m

    in_maps = []
    for core in range(NCORES):
        cm, xh = core // 2, core % 2
        resid = np.empty((2, 2, C, RROWS, RCOLS), bfloat16)
        for j in range(2):
            resid[:, j] = xp[:, cm:cm + RROWS,
                             xh * 128 + j:xh * 128 + j + RCOLS, :].transpose(
                                 0, 3, 1, 2)
        resid = resid.reshape(2, 128, RSZ)

        wc = np.zeros((2, totw), np.float32)        # [j, totw] win weights
        wg = np.zeros((2, 2, totng), np.float32)    # [j, i, totng] wrap w
        idx_c = np.zeros(max(totng, 256), np.int16)
        xls = np.arange(XW)
        gxs = xh * 128 + xls
        for ch in plan:
            ci = ch["ci"]
            h = 4 * ci + cm
            wo = ch["wfree"] and 0
            wo = 0
            for t in ch["win"]:
                k, a0, aw_, b0, bw_ = (t["k"], t["amin"], t["awid"],
                                       t["bmin"], t["bwid"])
                wjt = np.zeros((2, aw_, bw_ - 1, XW), np.float32)
                dyv = y0a[h, gxs, k] - h - a0
                bsv = x0a[h, gxs, k] - gxs - b0
                for i in range(2):
                    a = dyv + i
                    ok = (a >= 0) & (a < aw_) & (bsv >= 0) & (bsv <= bw_ - 2)
                    ii = np.where(ok)[0]
                    for jj in range(2):
                        wjt[jj, a[ii], bsv[ii], ii] = wts[h, gxs[ii], k, i, jj]
                base = int(np.sum([c2["wfree"] for c2 in plan[:ci]]))
                wc[:, base + t["soff"] * XW:
                   base + (t["soff"] + t["nslot"]) * XW] = (
                    wjt.reshape(2, t["nslot"] * XW))
            go = int(goffs[ch["grp"]]) + ch["goff"]
            for t in ch["wrap"]:
                k, lo, wd, noff = t["k"], t["lo"], t["wd"], t["noff"]
                gcl = xh * 128 + np.arange(lo, lo + wd)
                valid = gcl < W
                gc = np.clip(gcl, 0, W - 1)
                idx_c[go + noff:go + noff + wd] = np.where(
                    valid, pidx[h, gc, k], 0)
                for i in range(2):
                    for jj in range(2):
                        wv = np.where(valid, wcell[h, gc, k, i, jj], 0.0)
                        wg[jj, i, go + noff:go + noff + wd] = wv
        pp = np.arange(128)
        wfull = wc[pp // 64, :].astype(bfloat16)
        # wgr layout per group: [128, (i2, ngg)] regions concatenated
        wgfull = np.zeros((128, 2 * totng), np.float32)
        for gr in groups:
            g, ngg = gr["g"], gr["ngg"]
            if not ngg:
                continue
            go = int(goffs[g])
            blk = wg[:, :, go:go + ngg]            # [j, i, ngg]
            wgfull[:, 2 * go:2 * (go + ngg)] = (
                blk[pp // 64].reshape(128, 2 * ngg))
        n16 = max(totng // 16, 16)
        iw = idx_c[:n16 * 16].reshape(n16, 16).T.astype(np.int16)
        in_maps.append({
            "resid": resid,
            "wcomp": wfull,
            "wgr": wgfull.astype(bfloat16),
            "widx": np.tile(iw, (8, 1)),
            "xdp": xdp,
            "kdup": kdup,
        })
    return in_maps


_CACHE = {}
LAST_EXEC_NS = None


def kernel(x, kernel, scale, offset_base):
    global LAST_EXEC_NS
    x = np.asarray(x, np.float32)
    kern = np.asarray(kernel, np.float32)
    scale = np.asarray(scale, np.float32)
    offset_base = np.asarray(offset_base, np.float32)

    y0a, x0a, wts = geometry(scale, offset_base)
    plan, groups, order = make_plan(y0a, x0a, wts)
    sig = plan_sig(plan, groups, order)
    if sig not in _CACHE:
        _CACHE[sig] = build_bass(plan, groups, order)
    nc = _CACHE[sig]

    in_maps = pack_inputs(x, kern, plan, groups, y0a, x0a, wts)

    import os, sys, types
    trace = bool(os.environ.get("CHCONV_TRACE"))
    if trace:
        try:
            import antenv.axon_hooks  # noqa: F401
        except ImportError:
            from trn_agent_boot.trn_boot import _ntff_profile_via_ctypes
            hook = _ntff_profile_via_ctypes("/opt/axon/libaxon_pjrt.so")
            mod = types.ModuleType("antenv.axon_hooks")
            mod.get_axon_ntff_profile_hook = lambda: hook
            sys.modules["antenv.axon_hooks"] = mod
    res = run_bass_kernel_spmd(nc, in_maps, core_ids=list(range(NCORES)),
                               trace=trace)
    LAST_EXEC_NS = res.exec_time_ns

    out = np.empty((B, H, W, F), np.float32)
    for core in range(NCORES):
        cm, xh = core // 2, core % 2
        o = np.asarray(res.results[core]["out"], np.float32)
        o = o.reshape(F, NCH, 2, XW)
        for m in range(B):
            out[m, cm::4, xh * 128:xh * 128 + XW, :] = (
                o[:, :, m, :].transpose(1, 2, 0))
    return out


# revision 48
# speedup vs baseline: 1.2568x; 1.2568x over previous
"""Trainium2 Bass kernel for nn_CHConv — windowed deformable 3x3 conv, v5.

Design notes:
- Geometry (offsets/windows/weights) is image-independent. Shard by
  (cm = h%4, xh = x-half): core = cm*2+xh handles rows {4j+cm}, columns
  [xh*128, xh*128+128), for BOTH images. The two images share one weight
  read (halves the dominant weight DMA vs batch-sharding).
- The core's column slice of BOTH images stays resident in SBUF,
  j-duplicated: partition (j*64+c) holds x[img, row, col+j, c] for a
  [136 row x 146 col] padded window. One chunk = one row-group; windowed
  taps read strided windows straight from the resident tile.
- T-build: one 5-dim DVE tensor_tensor per tap covers both images
  (weights read once, img dim stride-0 on the weight AP).
- Seam taps (equirect wrap, dx ~ +204..255) use gpsimd.dma_gather with
  1024B elements carrying BOTH images' 2x2xC patch; gathers are grouped
  8 chunks per call (padding only per group) to cut SWDGE descriptor
  generation cost.
- Matmuls: k-major accumulation into one PSUM bank [128F, 256=(img,x)]
  per chunk; wrap matmuls land on column intervals. ACT copies PSUM out
  as bf16.
"""
import numpy as np
from contextlib import ExitStack

import concourse.bass as bass
import concourse.bacc as bacc
import concourse.mybir as mybir
import concourse.tile as tile
from concourse.bass_utils import run_bass_kernel_spmd
from ml_dtypes import bfloat16

H, W, K, C, F, B = 128, 256, 9, 64, 128, 2
NCH = 32            # chunks (row-groups) per core
GSZ = 8             # chunks per gather group
NG = NCH // GSZ     # gather groups
XW = 128            # x columns per core
RROWS = 136         # resident rows: image rows [cm-4, cm+132)
RCOLS = 146         # resident cols: image cols [xh*128-8, xh*128+138)
HALO = 8
A_MAX = 10
NPIX = H * W
NCORES = 8
RSZ = RROWS * RCOLS

_BF16 = mybir.dt.bfloat16
_F32 = mybir.dt.float32
_I16 = mybir.dt.int16


# ---------------------------------------------------------------- host plan
def geometry(scale, offset_base):
    off = (offset_base.astype(np.float32) * scale.astype(np.float32)).reshape(
        H, W, K, 2)
    ti, tj = np.meshgrid(np.arange(3), np.arange(3), indexing="ij")
    ti = ti.reshape(-1).astype(np.float32)
    tj = tj.reshape(-1).astype(np.float32)
    ys = (np.arange(H, dtype=np.float32)[:, None, None] - 1.0 + ti[None, None]
          + off[..., 0])
    xs = (np.arange(W, dtype=np.float32)[None, :, None] - 1.0 + tj[None, None]
          + off[..., 1])
    y0 = np.floor(ys); x0 = np.floor(xs)
    fy = (ys - y0).astype(np.float32); fx = (xs - x0).astype(np.float32)
    y0 = y0.astype(np.int64); x0 = x0.astype(np.int64)

    def v(yi, xi):
        return (((yi >= 0) & (yi < H) & (xi >= 0) & (xi < W))
                .astype(np.float32))
    w = np.zeros((H, W, K, 2, 2), np.float32)
    w[..., 0, 0] = (1 - fy) * (1 - fx) * v(y0, x0)
    w[..., 0, 1] = (1 - fy) * fx * v(y0, x0 + 1)
    w[..., 1, 0] = fy * (1 - fx) * v(y0 + 1, x0)
    w[..., 1, 1] = fy * fx * v(y0 + 1, x0 + 1)
    return y0, x0, w


def make_plan(y0, x0, w):
    """Per-chunk meta shared by all 8 cores (union over 4 rows x 2 halves)."""
    live = w.sum(axis=(3, 4)) > 0
    dy = y0 - np.arange(H)[:, None, None]
    dx = x0 - np.arange(W)[None, :, None]
    plan = []
    for ci in range(NCH):
        rows = [4 * ci + r for r in range(4)]
        win, wrap = [], []
        for k in range(K):
            lv = np.stack([live[h, :, k] for h in rows])
            if not lv.any():
                continue
            dyr = np.stack([dy[h, :, k] for h in rows])
            dxr = np.stack([dx[h, :, k] for h in rows])
            dmn, dmx = int(dxr[lv].min()), int(dxr[lv].max())
            ymn, ymx = int(dyr[lv].min()), int(dyr[lv].max())
            awid, bwid = ymx - ymn + 2, dmx - dmn + 2
            amin, bmin = ymn, dmn
            ok = (bmin >= -HALO and bmin + bwid <= HALO + 3
                  and awid <= A_MAX
                  and 4 * ci + 4 + amin >= 0
                  and 4 * ci + 4 + amin + awid <= RROWS)
            if ok:
                win.append(dict(k=k, amin=amin, awid=awid, bmin=bmin,
                                bwid=bwid, nslot=awid * (bwid - 1)))
            else:
                ws = np.where(lv.any(axis=0))[0]
                glo, ghi = int(ws.min()), int(ws.max())
                lo, hi = 127, 0
                for xh in range(2):
                    l_ = max(glo - 128 * xh, 0)
                    h_ = min(ghi - 128 * xh, 127)
                    if l_ <= h_:
                        lo = min(lo, l_); hi = max(hi, h_)
                assert lo <= hi
                wrap.append(dict(k=k, lo=lo, wd=hi - lo + 1))
        assert win, f"chunk {ci} has no windowed tap"
        soff = 0
        for t in win:
            t["soff"] = soff
            soff += t["nslot"]
        nw = sum(t["wd"] for t in wrap)
        noff = 0
        for t in wrap:
            t["noff"] = noff
            noff += t["wd"]
        plan.append(dict(ci=ci, win=win, wrap=wrap, nslots=soff, nw=nw,
                         wfree=soff * XW))
    # processing order: interior chunks first so the gather-heavy pole
    # chunks (0-2) overlap with the bulk of the compute.
    interior = [ci for ci in range(NCH) if plan[ci]["nw"] <= 400]
    poles = [ci for ci in range(NCH) if plan[ci]["nw"] > 400]
    cut = (2 * len(interior)) // 3
    order = interior[:cut] + poles + interior[cut:]
    # gather groups: adaptive over processing order — big-gather chunks go
    # solo, interior chunks pack greedily up to ~1200 idxs.
    groups = []
    cur, cur_nw = [], 0
    for ci in order:
        ch = plan[ci]
        big = ch["nw"] > 400
        if cur and (big or cur_nw + ch["nw"] > 450):
            groups.append(cur)
            cur, cur_nw = [], 0
        cur.append(ci)
        cur_nw += ch["nw"]
        if big or len(cur) >= 3:
            groups.append(cur)
            cur, cur_nw = [], 0
    if cur:
        groups.append(cur)
    gmeta = []
    for g, chs in enumerate(groups):
        off = 0
        for ci in chs:
            plan[ci]["goff"] = off
            plan[ci]["grp"] = g
            off += plan[ci]["nw"]
        ngg = ((off + 127) // 128) * 128 if off else 0
        gmeta.append(dict(g=g, ngg=ngg, raw=off, chunks=tuple(chs)))
    return plan, gmeta, order


def plan_sig(plan, groups, order):
    sig = []
    for ch in plan:
        sig.append((ch["nslots"], ch["nw"], ch["goff"], ch["grp"],
                    tuple((t["k"], t["amin"], t["awid"], t["bmin"], t["bwid"])
                          for t in ch["win"]),
                    tuple((t["k"], t["lo"], t["wd"]) for t in ch["wrap"])))
    sig.append(tuple((gr["ngg"], gr["chunks"]) for gr in groups))
    sig.append(tuple(order))
    return tuple(sig)


# ---------------------------------------------------------------- bass build
def build_bass(plan, groups, order):
    totw = sum(ch["wfree"] for ch in plan)
    totng = sum(gr["ngg"] for gr in groups)
    goffs = np.cumsum([0] + [gr["ngg"] for gr in groups])[:-1]

    nc = bacc.Bacc("TRN2", target_bir_lowering=False, debug=False)
    resid = nc.dram_tensor("resid", [2, 128, RSZ], _BF16,
                           kind="ExternalInput")          # [img, (j,c), r*c]
    wcomp = nc.dram_tensor("wcomp", [128, totw], _BF16, kind="ExternalInput")
    wgr = nc.dram_tensor("wgr", [128, 2 * totng], _BF16,
                         kind="ExternalInput")            # per-group [i2, ngg]
    widx = nc.dram_tensor("widx", [128, max(totng // 16, 16)], _I16,
                          kind="ExternalInput")
    xdp = nc.dram_tensor("xdp", [NPIX, 8 * C], _BF16, kind="ExternalInput")
    kdup = nc.dram_tensor("kdup", [128, K * F], _BF16, kind="ExternalInput")
    out = nc.dram_tensor("out", [F, NCH * 256], _BF16, kind="ExternalOutput")

    woffs = np.cumsum([0] + [ch["wfree"] for ch in plan])[:-1]

    with ExitStack() as ctx:
        tc = ctx.enter_context(tile.TileContext(nc))
        rp = ctx.enter_context(tc.tile_pool(name="rp", bufs=1))
        kp = ctx.enter_context(tc.tile_pool(name="kp", bufs=1))
        idxp = ctx.enter_context(tc.tile_pool(name="idxp", bufs=1))
        wbp = ctx.enter_context(tc.tile_pool(name="wbp", bufs=5))
        wgp = ctx.enter_context(tc.tile_pool(name="wgp", bufs=3))
        twp = ctx.enter_context(tc.tile_pool(name="twp", bufs=3))
        gtp = ctx.enter_context(tc.tile_pool(name="gtp", bufs=5))
        trp = ctx.enter_context(tc.tile_pool(name="trp", bufs=1))
        op_ = ctx.enter_context(tc.tile_pool(name="op", bufs=3))
        max_open = max(
            len(groups[i]["chunks"]) + (len(groups[i + 1]["chunks"])
                                        if i + 1 < len(groups) else 0)
            for i in range(len(groups)))
        psp = ctx.enter_context(tc.tile_pool(name="psp",
                                             bufs=min(max_open + 1, 8),
                                             space="PSUM"))

        kd = kp.tile([128, K * F], _BF16)
        nc.sync.dma_start(out=kd[:], in_=kdup[:, :])
        idx_t = idxp.tile([128, totng // 16], _I16)
        nc.sync.dma_start(out=idx_t[:], in_=widx[:, 0:totng // 16])
        res = rp.tile([128, 2 * RSZ], _BF16)
        # band-split resident load (17-row bands) so early chunks can start
        # as soon as their rows land; bands follow chunk processing order.
        BAND = 17
        nbands = (RROWS + BAND - 1) // BAND
        for b in range(nbands):
            r0, r1 = b * BAND, min((b + 1) * BAND, RROWS)
            for m in range(2):
                nc.sync.dma_start(
                    out=res[:, m * RSZ + r0 * RCOLS:m * RSZ + r1 * RCOLS],
                    in_=resid[m, :, r0 * RCOLS:r1 * RCOLS])

        wbs, gts, wgs, trs = {}, {}, {}, {}

        def stage_load(t):
            ch = plan[t]
            wbs[t] = wbp.tile([128, ch["wfree"]], _BF16, name=f"wb{t}",
                              tag="wb")
            nc.scalar.dma_start(out=wbs[t][:],
                                in_=wcomp[:, woffs[t]:woffs[t] + ch["wfree"]])

        def stage_gather(g):
            ngg = groups[g]["ngg"]
            if not ngg:
                return
            gts[g] = gtp.tile([128, 4, ngg], _BF16, name=f"gt{g}", tag="gt")
            wgs[g] = wgp.tile([128, 2, ngg], _BF16, name=f"wg{g}", tag="wg")
            go = int(goffs[g])
            nc.scalar.dma_start(out=wgs[g][:],
                                in_=wgr[:, 2 * go:2 * (go + ngg)])
            nc.gpsimd.dma_gather(
                out_ap=gts[g][:], in_ap=xdp[:, :],
                idxs_ap=idx_t[:, go // 16:(go + ngg) // 16],
                num_idxs=ngg, num_idxs_reg=ngg,
                elem_size=8 * C, transpose=True, single_packet=False,
            )

        def stage_wrap_weight(g):
            ngg = groups[g]["ngg"]
            if not ngg:
                return
            gt, wg = gts.pop(g), wgs.pop(g)
            t4 = trp.tile([128, 4, ngg], _BF16, name=f"t4{g}", tag="t4")
            w4 = bass.AP(wg.tensor, wg.offset,
                         [[wg.ap[0][0], 128], [0, 2], [ngg, 2], [1, ngg]])
            nc.vector.tensor_tensor(out=t4[:], in0=gt[:], in1=w4,
                                    op=mybir.AluOpType.mult)
            tr = trp.tile([128, 2, ngg], _BF16, name=f"tr{g}", tag="tr")
            t4v = t4[:].rearrange("p (m i) n -> p m i n", m=2)
            nc.vector.tensor_tensor(
                out=tr[:], in0=t4v[:, :, 0, :], in1=t4v[:, :, 1, :],
                op=mybir.AluOpType.add)
            trs[g] = tr

        ps_open = {}

        def stage_win(t):
            ch = plan[t]
            wb = wbs.pop(t)
            nslots = ch["nslots"]
            tw = twp.tile([128, nslots, 256], _BF16, name=f"tw{t}", tag="tw")
            ps = psp.tile([128, 256], _F32, space="PSUM", name=f"ps{t}",
                          tag="ps")
            ps_open[t] = ps
            ps = ps[:]

            rbase = res[:]
            for tp in ch["win"]:
                a0, b0 = tp["amin"], tp["bmin"]
                aw_, bw_ = tp["awid"], tp["bwid"]
                s0 = tp["soff"]
                if bw_ == 2:
                    src4 = bass.AP(
                        rbase.tensor,
                        rbase.offset + (4 * t + 4 + a0) * RCOLS + HALO + b0,
                        [[rbase.ap[0][0], 128], [RCOLS, aw_], [RSZ, 2],
                         [1, XW]],
                    )
                    wsl = bass.AP(
                        wb.tensor,
                        wb.offset + s0 * XW,
                        [[wb.ap[0][0], 128], [XW, aw_], [0, 2], [1, XW]],
                    )
                    dst4 = bass.AP(
                        tw.tensor,
                        tw.offset + s0 * 256,
                        [[tw.ap[0][0], 128], [256, aw_], [XW, 2], [1, XW]],
                    )
                    nc.vector.tensor_tensor(out=dst4, in0=src4, in1=wsl,
                                            op=mybir.AluOpType.mult)
                    continue
                wsl = bass.AP(
                    wb.tensor,
                    wb.offset + s0 * XW,
                    [[wb.ap[0][0], 128], [XW * (bw_ - 1), aw_],
                     [XW, bw_ - 1], [1, XW]],
                )
                for m in range(2):
                    src4 = bass.AP(
                        rbase.tensor,
                        rbase.offset + m * RSZ
                        + (4 * t + 4 + a0) * RCOLS + HALO + b0,
                        [[rbase.ap[0][0], 128], [RCOLS, aw_], [1, bw_ - 1],
                         [1, XW]],
                    )
                    dst4 = bass.AP(
                        tw.tensor,
                        tw.offset + s0 * 256 + m * XW,
                        [[tw.ap[0][0], 128], [256 * (bw_ - 1), aw_],
                         [256, bw_ - 1], [1, XW]],
                    )
                    nc.vector.tensor_tensor(out=dst4, in0=src4, in1=wsl,
                                            op=mybir.AluOpType.mult)

            mm_list = []
            for tp in ch["win"]:
                for sx in range(tp["nslot"]):
                    mm_list.append((tp["k"], tp["soff"] + sx))
            mm_list.sort()
            has_wrap = bool(ch["wrap"])
            last = len(mm_list) - 1
            for i, (k, s) in enumerate(mm_list):
                nc.tensor.matmul(
                    ps, lhsT=kd[:, k * F:(k + 1) * F],
                    rhs=tw[:, s, :], start=(i == 0),
                    stop=(i == last and not has_wrap))

        def finish_group(g):
            stage_wrap_weight(g)
            tr = trs.pop(g, None)
            chs = groups[g]["chunks"]
            for idx, t in enumerate(chs):
                ch = plan[t]
                pst = ps_open.pop(t)
                ps = pst[:]
                goff_c = ch.get("goff", 0)
                wr_list = []
                for tp in ch["wrap"]:
                    for m in range(2):
                        wr_list.append((tp["k"], tp["lo"], tp["wd"],
                                        tp["noff"], m))
                last = len(wr_list) - 1
                for i, (k, lo, wd, noff, m) in enumerate(wr_list):
                    nc.tensor.matmul(
                        ps[:, m * XW + lo:m * XW + lo + wd],
                        lhsT=kd[:, k * F:(k + 1) * F],
                        rhs=tr[:, m, goff_c + noff:goff_c + noff + wd],
                        start=False, stop=(i == last))
                ob = op_.tile([128, 256], _BF16, name=f"ob{t}", tag="ob")
                nc.scalar.copy(out=ob[:], in_=pst[:])
                nc.scalar.dma_start(out=out[:, t * 256:(t + 1) * 256],
                                    in_=ob[:])

        for i in range(min(5, NCH)):
            stage_load(order[i])
        for g in range(min(3, len(groups))):
            stage_gather(g)
        pos = 0
        for gi, gr in enumerate(groups):
            if gi + 3 < len(groups):
                stage_gather(gi + 3)
            for t in gr["chunks"]:
                if pos + 5 < NCH:
                    stage_load(order[pos + 5])
                stage_win(t)
                pos += 1
            if gi >= 1:
                finish_group(gi - 1)
        finish_group(len(groups) - 1)
    nc.finalize()
    return nc


# ---------------------------------------------------------------- host pack
def pack_inputs(x, kern, plan, groups, y0a, x0a, wts):
    xbf = x.astype(np.float32)
    xp = np.zeros((B, 140, 275, C), np.float32)
    xp[:, 4:4 + H, 8:8 + W, :] = xbf
    xp = xp.astype(bfloat16)

    xpad = np.pad(xbf, [(0, 0), (0, 1), (0, 1), (0, 0)])
    xd = np.empty((H, W, 2, 2, 2, C), np.float32)
    for i in range(2):
        for j in range(2):
            xd[:, :, :, i, j, :] = xpad[:, i:i + H, j:j + W, :].transpose(
                1, 2, 0, 3)
    xdp = xd.reshape(NPIX, 8 * C).astype(bfloat16)

    km = kern.reshape(F, C, K).astype(np.float32)
    kdv = km.transpose(1, 2, 0).reshape(C, K * F)
    kdup = np.concatenate([kdv, kdv], axis=0).astype(bfloat16)

    totw = sum(ch["wfree"] for ch in plan)
    totng = sum(gr["ngg"] for gr in groups)
    goffs = np.cumsum([0] + [gr["ngg"] for gr in groups])[:-1]

    gy = np.clip(y0a, 0, H - 2)
    gx = np.clip(x0a, 0, W - 2)
    pidx = (gy * W + gx).astype(np.int32)
    wcell = np.zeros((H, W, K, 2, 2), np.float32)
    for a_ in range(2):
        for b_ in range(2):
            for i in range(2):
                for j in range(2):
                    m = ((y0a + a_) == (gy + i)) & ((x0a + b_) == (gx + j))
                    wcell[..., i, j] += wts[..., a_, b_] * m

    in_maps = []
    for core in range(NCORES):
        cm, xh = core // 2, core % 2
        resid = np.empty((2, 2, C, RROWS, RCOLS), bfloat16)
        for j in range(2):
            resid[:, j] = xp[:, cm:cm + RROWS,
                             xh * 128 + j:xh * 128 + j + RCOLS, :].transpose(
                                 0, 3, 1, 2)
        resid = resid.reshape(2, 128, RSZ)

        wc = np.zeros((2, totw), np.float32)        # [j, totw] win weights
        wg = np.zeros((2, 2, totng), np.float32)    # [j, i, totng] wrap w
        idx_c = np.zeros(max(totng, 256), np.int16)
        xls = np.arange(XW)
        gxs = xh * 128 + xls
        for ch in plan:
            ci = ch["ci"]
            h = 4 * ci + cm
            wo = ch["wfree"] and 0
            wo = 0
            for t in ch["win"]:
                k, a0, aw_, b0, bw_ = (t["k"], t["amin"], t["awid"],
                                       t["bmin"], t["bwid"])
                wjt = np.zeros((2, aw_, bw_ - 1, XW), np.float32)
                dyv = y0a[h, gxs, k] - h - a0
                bsv = x0a[h, gxs, k] - gxs - b0
                for i in range(2):
                    a = dyv + i
                    ok = (a >= 0) & (a < aw_) & (bsv >= 0) & (bsv <= bw_ - 2)
                    ii = np.where(ok)[0]
                    for jj in range(2):
                        wjt[jj, a[ii], bsv[ii], ii] = wts[h, gxs[ii], k, i, jj]
                base = int(np.sum([c2["wfree"] for c2 in plan[:ci]]))
                wc[:, base + t["soff"] * XW:
                   base + (t["soff"] + t["nslot"]) * XW] = (
                    wjt.reshape(2, t["nslot"] * XW))
            go = int(goffs[ch["grp"]]) + ch["goff"]
            for t in ch["wrap"]:
                k, lo, wd, noff = t["k"], t["lo"], t["wd"], t["noff"]
                gcl = xh * 128 + np.arange(lo, lo + wd)
                valid = gcl < W
                gc = np.clip(gcl, 0, W - 1)
                idx_c[go + noff:go + noff + wd] = np.where(
                    valid, pidx[h, gc, k], 0)
                for i in range(2):
                    for jj in range(2):
                        wv = np.where(valid, wcell[h, gc, k, i, jj], 0.0)
                        wg[jj, i, go + noff:go + noff + wd] = wv
        pp = np.arange(128)
        wfull = wc[pp // 64, :].astype(bfloat16)
        # wgr layout per group: [128, (i2, ngg)] regions concatenated
        wgfull = np.zeros((128, 2 * totng), np.float32)
        for gr in groups:
            g, ngg = gr["g"], gr["ngg"]
            if not ngg:
                continue
            go = int(goffs[g])
            blk = wg[:, :, go:go + ngg]            # [j, i, ngg]
            wgfull[:, 2 * go:2 * (go + ngg)] = (
                blk[pp // 64].reshape(128, 2 * ngg))
        n16 = max(totng // 16, 16)
        iw = idx_c[:n16 * 16].reshape(n16, 16).T.astype(np.int16)
        in_maps.append({
            "resid": resid,
            "wcomp": wfull,
            "wgr": wgfull.astype(bfloat16),
            "widx": np.tile(iw, (8, 1)),
            "xdp": xdp,
            "kdup": kdup,
        })
    return in_maps


_CACHE = {}
LAST_EXEC_NS = None


def kernel(x, kernel, scale, offset_base):
    global LAST_EXEC_NS
    x = np.asarray(x, np.float32)
    kern = np.asarray(kernel, np.float32)
    scale = np.asarray(scale, np.float32)
    offset_base = np.asarray(offset_base, np.float32)

    y0a, x0a, wts = geometry(scale, offset_base)
    plan, groups, order = make_plan(y0a, x0a, wts)
    sig = plan_sig(plan, groups, order)
    if sig not in _CACHE:
        _CACHE[sig] = build_bass(plan, groups, order)
    nc = _CACHE[sig]

    in_maps = pack_inputs(x, kern, plan, groups, y0a, x0a, wts)

    import os, sys, types
    trace = bool(os.environ.get("CHCONV_TRACE"))
    if trace:
        try:
            import antenv.axon_hooks  # noqa: F401
        except ImportError:
            from trn_agent_boot.trn_boot import _ntff_profile_via_ctypes
            hook = _ntff_profile_via_ctypes("/opt/axon/libaxon_pjrt.so")
            mod = types.ModuleType("antenv.axon_hooks")
            mod.get_axon_ntff_profile_hook = lambda: hook
            sys.modules["antenv.axon_hooks"] = mod
    res = run_bass_kernel_spmd(nc, in_maps, core_ids=list(range(NCORES)),
                               trace=trace)
    LAST_EXEC_NS = res.exec_time_ns

    out = np.empty((B, H, W, F), np.float32)
    for core in range(NCORES):
        cm, xh = core // 2, core % 2
        o = np.asarray(res.results[core]["out"], np.float32)
        o = o.reshape(F, NCH, 2, XW)
        for m in range(B):
            out[m, cm::4, xh * 128:xh * 128 + XW, :] = (
                o[:, :, m, :].transpose(1, 2, 0))
    return out
